# revision 5
# baseline (speedup 1.0000x reference)
"""Trainium2 Bass kernel for nn_RX_2448131359093.

reference math: phi' = FWHT(FWHT(phi) * exp(-0.5i * thetas @ BITS))
with phi complex64 [512, 16384], FWHT over the last dim (2^14), batch rows
independent -> pure data parallel over 8 NeuronCores (64 rows/core).

Device kernel (per core, fp16 data path, see build_program):
  - 16384-point FWHT via H_128 (x) H_128: per 128x128 row-block X,
    V = mm(lhsT=X, rhs=H) = X^T H, A = mm(lhsT=V, rhs=Hs) = 2^-7 H X H.
    Using the data as the stationary operand flips orientation each
    matmul, so no PE transposes are needed.
  - RZ phase is separable: E[i,j] = u(i) v(j), built on the PE from
    16-lane rank tables; the complex multiply B = A*E is four partial
    products whose add/sub is folded into the second FWHT's PSUM
    accumulation (paired start/stop matmuls).
  - Inputs/outputs are pre/post-transposed on the host to [i, (r j)]
    so every DMA descriptor is a contiguous 16KB partition row.

Dispatch: the jax/PJRT wrapper (shard_map over 8 axon devices) is built
ONCE and cached; constants and the ignored output-slot operands are
device-resident. Per call only phi (fp16, 32MB) + phase tables go up and
fp16 output planes come down.
"""
import sys

sys.path.insert(0, "/opt/trn_rl_repo")
import numpy as np
import concourse.bacc as bacc
import concourse.bass as bass
import concourse.mybir as mybir
import concourse.tile as tile
from concourse.alu_op_type import AluOpType

F32 = mybir.dt.float32
F16 = mybir.dt.float16

B = 512
S = 16384
N_CORES = 8
RPC = B // N_CORES       # 64 rows per core
CHUNKS = 8
RPCH = RPC // CHUNKS     # 8 rows per chunk
CW = RPCH * 128          # 1024 chunk width
W = RPC * 128            # 8192 free width of resident planes

_cache = {}


def _hadamard128():
    idx = np.arange(128)
    m = idx[:, None] & idx[None, :]
    par = np.zeros_like(m)
    for b in range(7):
        par ^= (m >> b) & 1
    return np.where(par == 0, 1.0, -1.0)


def _bits7():
    q = np.arange(7)
    i = np.arange(128)
    return ((i[None, :] >> (6 - q)[:, None]) & 1).astype(np.float64)


def build_program():
    nc = bacc.Bacc("TRN2", target_bir_lowering=False, debug=False)
    d_xre = nc.dram_tensor("xre", [128, W], F16, kind="ExternalInput").ap()
    d_xim = nc.dram_tensor("xim", [128, W], F16, kind="ExternalInput").ap()
    d_h = nc.dram_tensor("h", [128, 128], F16, kind="ExternalInput").ap()
    d_hs = nc.dram_tensor("hs", [128, 128], F16, kind="ExternalInput").ap()
    d_hn = nc.dram_tensor("hn", [128, 128], F16, kind="ExternalInput").ap()
    d_lure = nc.dram_tensor("lure", [16, CHUNKS * 128], F16, kind="ExternalInput").ap()
    d_luim = nc.dram_tensor("luim", [16, CHUNKS * 128], F16, kind="ExternalInput").ap()
    d_rv = nc.dram_tensor("rv", [16, CHUNKS * CW], F16, kind="ExternalInput").ap()
    d_ore = nc.dram_tensor("ore", [128, W], F16, kind="ExternalOutput").ap()
    d_oim = nc.dram_tensor("oim", [128, W], F16, kind="ExternalOutput").ap()

    with tile.TileContext(nc) as tc:
        with tc.tile_pool(name="const", bufs=1) as cp, \
             tc.tile_pool(name="big", bufs=1) as bigp, \
             tc.tile_pool(name="work", bufs=3) as wp, \
             tc.tile_pool(name="ps", bufs=4, space=bass.MemorySpace.PSUM) as psp:

            t_h = cp.tile([128, 128], F16, name="t_h")
            t_hs = cp.tile([128, 128], F16, name="t_hs")
            t_hn = cp.tile([128, 128], F16, name="t_hn")
            t_lure = cp.tile([16, CHUNKS * 128], F16, name="t_lure")
            t_luim = cp.tile([16, CHUNKS * 128], F16, name="t_luim")
            t_rv = cp.tile([16, CHUNKS * CW], F16, name="t_rv")
            # DMA issue is serialized per issuing engine; tables go first so
            # the inline E matmuls can start as soon as the PE boots.
            for t, d in [(t_h, d_h), (t_hs, d_hs), (t_hn, d_hn),
                         (t_lure, d_lure),
                         (t_luim, d_luim), (t_rv, d_rv)]:
                nc.sync.dma_start(t[:], d)

            t_xre = bigp.tile([128, W], F16, name="t_xre")
            t_xim = bigp.tile([128, W], F16, name="t_xim")
            t_ore = bigp.tile([128, W], F16, name="t_ore")
            t_oim = bigp.tile([128, W], F16, name="t_oim")
            t_Ere = bigp.tile([128, W], F16, name="t_Ere")
            t_Eim = bigp.tile([128, W], F16, name="t_Eim")
            # Input planes quarter-split on TWO issuing engines -> two HW DMA
            # queues run in parallel and the issue itself is 4 instrs/plane.
            for q in range(4):
                qs = slice(q * (W // 4), (q + 1) * (W // 4))
                nc.sync.dma_start(t_xre[:, qs], d_xre[:, qs])
                nc.gpsimd.dma_start(t_xim[:, qs], d_xim[:, qs])

            def ps_tile():
                return psp.tile([128, CW], F32, name="ps", tag="ps")

            # ---- all E tiles upfront (input-independent, overlaps the
            # input DMA window where the copy engines are otherwise idle) ----
            for c in range(CHUNKS):
                cs = slice(c * CW, (c + 1) * CW)
                ls = slice(c * 128, (c + 1) * 128)
                pEre = ps_tile()
                pEim = ps_tile()
                for hh in range(2):
                    hs_ = slice(hh * 512, (hh + 1) * 512)
                    cs2 = slice(c * CW + hh * 512, c * CW + (hh + 1) * 512)
                    nc.tensor.matmul(pEre[:, hs_], t_lure[:, ls],
                                     t_rv[:, cs2], start=True, stop=True)
                    nc.tensor.matmul(pEim[:, hs_], t_luim[:, ls],
                                     t_rv[:, cs2], start=True, stop=True)
                nc.vector.tensor_copy(t_Ere[:, cs], pEre[:])
                nc.scalar.copy(t_Eim[:, cs], pEim[:])

            # Chunks processed in interleaved PAIRS: PSUM-pool FIFO stalls
            # then coincide with true dependencies, and each chunk's PE
            # stages fill the other's copy/product latencies.
            for cc in range(0, CHUNKS, 2):
                pair = (cc, cc + 1)
                pV, sV, pA, sA, P, pW, sW, pY = {}, {}, {}, {}, {}, {}, {}, {}

                for c in pair:
                    pV[c] = (ps_tile(), ps_tile())
                    for b in range(RPCH):
                        bs = slice(c * CW + b * 128, c * CW + (b + 1) * 128)
                        ps_b = slice(b * 128, (b + 1) * 128)
                        nc.tensor.matmul(pV[c][0][:, ps_b], t_xre[:, bs],
                                         t_h[:], start=True, stop=True)
                        nc.tensor.matmul(pV[c][1][:, ps_b], t_xim[:, bs],
                                         t_h[:], start=True, stop=True)
                for c in pair:
                    sV[c] = (wp.tile([128, CW], F16, name="sVre"),
                             wp.tile([128, CW], F16, name="sVim"))
                    nc.scalar.copy(sV[c][0][:], pV[c][0][:])
                    nc.vector.tensor_copy(sV[c][1][:], pV[c][1][:])
                for c in pair:
                    pA[c] = (ps_tile(), ps_tile())
                    for b in range(RPCH):
                        ps_b = slice(b * 128, (b + 1) * 128)
                        nc.tensor.matmul(pA[c][0][:, ps_b], sV[c][0][:, ps_b],
                                         t_hs[:], start=True, stop=True)
                        nc.tensor.matmul(pA[c][1][:, ps_b], sV[c][1][:, ps_b],
                                         t_hs[:], start=True, stop=True)
                for c in pair:
                    sA[c] = (wp.tile([128, CW], F16, name="sAre"),
                             wp.tile([128, CW], F16, name="sAim"))
                    nc.scalar.copy(sA[c][0][:], pA[c][0][:])
                    nc.scalar.copy(sA[c][1][:], pA[c][1][:])
                for c in pair:
                    cs = slice(c * CW, (c + 1) * CW)
                    # B = A * E with add/sub folded into FWHT #2 accumulation:
                    # W_re = (Are*Ere)^T H + ((-Aim)*Eim)^T H
                    # W_im = (Are*Eim)^T H + (Aim*Ere)^T H
                    P[c] = tuple(wp.tile([128, CW], F16, name=f"P{k}")
                                 for k in range(4))
                    # half-split across gpsimd/vector: each product tile's two
                    # halves land on different engines, so every P tile
                    # completes in half the wall-time and W matmuls start per
                    # half. P2's subtraction happens in the W matmul via the
                    # negated H moving operand (t_hn) -- no engine cost.
                    srcs = [(sA[c][0], t_Ere), (sA[c][1], t_Eim),
                            (sA[c][0], t_Eim), (sA[c][1], t_Ere)]
                    for k, (a_t, e_t) in enumerate(srcs):
                        for hh in range(2):
                            sl = slice(hh * 512, (hh + 1) * 512)
                            el = slice(c * CW + hh * 512, c * CW + (hh + 1) * 512)
                            eng = nc.gpsimd if (k + hh) % 2 == 0 else nc.vector
                            eng.tensor_mul(P[c][k][:, sl], a_t[:, sl],
                                           e_t[:, el])
                for c in pair:
                    pW[c] = (ps_tile(), ps_tile())
                    for b in range(RPCH):
                        ps_b = slice(b * 128, (b + 1) * 128)
                        nc.tensor.matmul(pW[c][0][:, ps_b], P[c][0][:, ps_b],
                                         t_h[:], start=True, stop=False)
                        nc.tensor.matmul(pW[c][0][:, ps_b], P[c][1][:, ps_b],
                                         t_hn[:], start=False, stop=True)
                        nc.tensor.matmul(pW[c][1][:, ps_b], P[c][2][:, ps_b],
                                         t_h[:], start=True, stop=False)
                        nc.tensor.matmul(pW[c][1][:, ps_b], P[c][3][:, ps_b],
                                         t_h[:], start=False, stop=True)
                for c in pair:
                    sW[c] = (wp.tile([128, CW], F16, name="sWre"),
                             wp.tile([128, CW], F16, name="sWim"))
                    nc.scalar.copy(sW[c][0][:], pW[c][0][:])
                    nc.vector.tensor_copy(sW[c][1][:], pW[c][1][:])
                for c in pair:
                    pY[c] = (ps_tile(), ps_tile())
                    for b in range(RPCH):
                        ps_b = slice(b * 128, (b + 1) * 128)
                        nc.tensor.matmul(pY[c][0][:, ps_b], sW[c][0][:, ps_b],
                                         t_hs[:], start=True, stop=True)
                        nc.tensor.matmul(pY[c][1][:, ps_b], sW[c][1][:, ps_b],
                                         t_hs[:], start=True, stop=True)
                for c in pair:
                    cs = slice(c * CW, (c + 1) * CW)
                    nc.scalar.copy(t_ore[:, cs], pY[c][0][:])
                    nc.vector.tensor_copy(t_oim[:, cs], pY[c][1][:])
                ps = slice(cc * CW, (cc + 2) * CW)
                nc.scalar.dma_start(d_ore[:, ps], t_ore[:, ps])
                nc.scalar.dma_start(d_oim[:, ps], t_oim[:, ps])

    nc.compile()
    return nc


def host_tables(thetas_core):
    """lu/rv tables, fp16: E[i,(r,j)] = Re/Im(u_r(i) v_r(j)), 16 rank lanes."""
    th = thetas_core.astype(np.float64)
    bits = _bits7()
    Pi = 0.5 * (th[:, 0:7] @ bits)    # [64, 128] phase over i
    Pj = 0.5 * (th[:, 7:14] @ bits)   # [64, 128] phase over j
    u_re, u_im = np.cos(Pi), -np.sin(Pi)
    v_re, v_im = np.cos(Pj), -np.sin(Pj)
    ure = u_re.reshape(CHUNKS, RPCH, 128)
    uim = u_im.reshape(CHUNKS, RPCH, 128)
    vre = v_re.reshape(CHUNKS, RPCH, 128)
    vim = v_im.reshape(CHUNKS, RPCH, 128)
    lure = np.zeros((16, CHUNKS, 128), np.float16)
    luim = np.zeros((16, CHUNKS, 128), np.float16)
    rv = np.zeros((16, CHUNKS, RPCH, 128), np.float16)
    for rl in range(RPCH):
        # E_re = u_re v_re - u_im v_im ; E_im = u_im v_re + u_re v_im
        lure[2 * rl + 0] = ure[:, rl, :]
        lure[2 * rl + 1] = -uim[:, rl, :]
        luim[2 * rl + 0] = uim[:, rl, :]
        luim[2 * rl + 1] = ure[:, rl, :]
        rv[2 * rl + 0, :, rl, :] = vre[:, rl, :]
        rv[2 * rl + 1, :, rl, :] = vim[:, rl, :]
    return (lure.reshape(16, CHUNKS * 128), luim.reshape(16, CHUNKS * 128),
            rv.reshape(16, CHUNKS * CW))


def get_runner():
    """Build the program + jitted SPMD executor once; cache for reuse."""
    if "runner" in _cache:
        return _cache["runner"]
    import jax
    from jax.sharding import Mesh, PartitionSpec, NamedSharding
    try:
        from jax.experimental.shard_map import shard_map
    except ImportError:
        from jax import shard_map
    from concourse.bass2jax import (
        _bass_exec_p, install_neuronx_cc_hook, partition_id_tensor)

    try:
        jax.config.update("jax_compilation_cache_dir", "/tmp/jax_comp_cache")
    except Exception:
        pass

    install_neuronx_cc_hook()
    nc = build_program()

    partition_name = (nc.partition_id_tensor.name
                      if nc.partition_id_tensor else None)
    in_names, out_names, out_avals = [], [], []
    for alloc in nc.m.functions[0].allocations:
        if not isinstance(alloc, mybir.MemoryLocationSet):
            continue
        name = alloc.memorylocations[0].name
        if alloc.kind == "ExternalInput":
            if name != partition_name:
                in_names.append(name)
        elif alloc.kind == "ExternalOutput":
            out_names.append(name)
            out_avals.append(jax.core.ShapedArray(
                tuple(alloc.tensor_shape), mybir.dt.np(alloc.dtype)))
    all_in_names = tuple(in_names) + tuple(out_names)
    if partition_name is not None:
        all_in_names = all_in_names + (partition_name,)
    n_params, n_outs = len(in_names), len(out_names)

    def _body(*args):
        operands = list(args)
        if partition_name is not None:
            operands.append(partition_id_tensor())
        outs = _bass_exec_p.bind(
            *operands,
            out_avals=tuple(out_avals),
            in_names=all_in_names,
            out_names=tuple(out_names),
            lowering_input_output_aliases=(),
            sim_require_finite=True,
            sim_require_nnan=True,
            nc=nc,
        )
        return tuple(outs)

    devices = jax.devices()[:N_CORES]
    mesh = Mesh(np.asarray(devices), ("core",))
    sharded = jax.jit(
        shard_map(_body, mesh=mesh,
                  in_specs=(PartitionSpec("core"),) * (n_params + n_outs),
                  out_specs=(PartitionSpec("core"),) * n_outs,
                  check_rep=False),
    )
    sh = NamedSharding(mesh, PartitionSpec("core"))

    # Constants and the output-slot operands are uploaded once. The kernel
    # writes every element of ore/oim, so the zero content is never read.
    H = _hadamard128()
    const_np = {
        "h": np.tile(H.astype(np.float16), (N_CORES, 1)),
        "hs": np.tile((H * (2.0 ** -7)).astype(np.float16), (N_CORES, 1)),
        "hn": np.tile((-H).astype(np.float16), (N_CORES, 1)),
    }
    dev_const = {k: jax.device_put(v, sh) for k, v in const_np.items()}
    dev_zeros = [jax.device_put(
        np.zeros((N_CORES * av.shape[0], *av.shape[1:]), av.dtype), sh)
        for av in out_avals]
    jax.block_until_ready(list(dev_const.values()) + dev_zeros)

    runner = dict(nc=nc, jax=jax, sharded=sharded, sh=sh,
                  in_names=in_names, out_names=out_names,
                  dev_const=dev_const, dev_zeros=dev_zeros)
    _cache["runner"] = runner
    return runner


def _pre(phi):
    """[512,16384] f32 -> per-core [i, (r j)] fp16 global [1024, 8192]."""
    return (phi.reshape(N_CORES, RPC, 128, 128).transpose(0, 2, 1, 3)
            .astype(np.float16).reshape(N_CORES * 128, W))


def device_args(runner, phi_real, phi_imag, thetas):
    """Upload per-call inputs; returns the positional args for sharded()."""
    jax = runner["jax"]
    sh = runner["sh"]
    d_xre = jax.device_put(_pre(phi_real), sh)
    d_xim = jax.device_put(_pre(phi_imag), sh)
    lures, luims, rvs = [], [], []
    for k in range(N_CORES):
        lure, luim, rv = host_tables(thetas[k * RPC:(k + 1) * RPC])
        lures.append(lure)
        luims.append(luim)
        rvs.append(rv)
    tb = {
        "lure": jax.device_put(np.concatenate(lures, 0), sh),
        "luim": jax.device_put(np.concatenate(luims, 0), sh),
        "rv": jax.device_put(np.concatenate(rvs, 0), sh),
    }
    dev_in = []
    for n in runner["in_names"]:
        if n == "xre":
            dev_in.append(d_xre)
        elif n == "xim":
            dev_in.append(d_xim)
        elif n in tb:
            dev_in.append(tb[n])
        else:
            dev_in.append(runner["dev_const"][n])
    return dev_in + runner["dev_zeros"]


def assemble(ore, oim):
    """fp16 [1024, 8192] planes -> complex64 [512, 16384]."""
    res = np.empty((B, S), np.complex64)
    rv = res.view(np.float32).reshape(B, S, 2)
    rv[..., 0] = (ore.reshape(N_CORES, 128, RPC, 128).transpose(0, 2, 1, 3)
                  .reshape(B, S))
    rv[..., 1] = (oim.reshape(N_CORES, 128, RPC, 128).transpose(0, 2, 1, 3)
                  .reshape(B, S))
    return res


def kernel(phi_real, phi_imag, thetas):
    phi_real = np.asarray(phi_real, dtype=np.float32)
    phi_imag = np.asarray(phi_imag, dtype=np.float32)
    thetas = np.asarray(thetas, dtype=np.float32)

    runner = get_runner()
    jax = runner["jax"]
    args = device_args(runner, phi_real, phi_imag, thetas)
    outs = runner["sharded"](*args)
    ore, oim = [np.asarray(jax.device_get(o)) for o in outs]
    return assemble(ore, oim)
